# revision 8
# baseline (speedup 1.0000x reference)
"""Trainium2 Bass kernel for nn_LuongAttention (S=2048, B=16, H=1024), 8 cores.

Math (matching the reference):
  energy[s,b,j] = tanh( h[b,:] @ W1[j,:] + hs[s,b,:] @ W2[j,:] + b_att[j] )
                  with W1 = W_att[:, :H], W2 = W_att[:, H:]
  score[s,b]    = energy[s,b,:] . v      (masked_fill ~mask -> -1e10)
  w             = softmax(score, axis=b) (the reference's "buggy" batch softmax)
  context[b,:]  = sum_s w[s,b] * hs[s,b,:]

Sharding: S is split across the 8 cores (256 rows each). The softmax over b is
per-s, hence fully core-local; each core emits a partial context (sum over its
s-range) and the host sums the 8 partials.

Per-core layout (host-prepared, fp16 operands for full-rate PE matmuls with
fp32 PSUM accumulation):
  xt  [H, 4096]   : hs-shard transposed, token t = b*256 + s_local (b-major)
  xn  [256,16,H]  : hs-shard natural layout (context phase, s on partitions)
  w2t/w1t [H, H]  : W2^T / W1^T (k on partitions)
  ht  [H, 16]     : hidden[0]^T
  batt [H, 1] f32, v [H, 1] f16, mask [2, 128, 16] u8
Output per core: out [16, H] f32 partial context.
"""
import numpy as np

S, B, H = 2048, 16, 1024
NCORES = 8
SL = S // NCORES          # 256 s rows per core
TOK = SL * B              # 4096 tokens per core
KT = H // 128             # 8 contraction tiles
JT = H // 128             # 8 output-feature tiles
NCH = TOK // 512          # 8 token chunks of 512 (= 2 batch rows per chunk)

_cached = {}


def _build_nc():
    import concourse.bass as bass
    import concourse.mybir as mybir
    import concourse.tile as tile
    import concourse.masks as masks

    F32 = mybir.dt.float32
    F16 = mybir.dt.float16
    U8 = mybir.dt.uint8
    AF = mybir.ActivationFunctionType
    ALU = mybir.AluOpType

    nc = bass.Bass()
    xt_d = nc.dram_tensor("xt", [H, TOK], F16, kind="ExternalInput")
    xn_d = nc.dram_tensor("xn", [SL, B, H], F16, kind="ExternalInput")
    w2t_d = nc.dram_tensor("w2t", [H, H], F16, kind="ExternalInput")
    w1t_d = nc.dram_tensor("w1t", [H, H], F16, kind="ExternalInput")
    ht_d = nc.dram_tensor("ht", [H, B], F16, kind="ExternalInput")
    batt_d = nc.dram_tensor("batt", [H, 1], F32, kind="ExternalInput")
    v_d = nc.dram_tensor("v", [H, 1], F16, kind="ExternalInput")
    mask_d = nc.dram_tensor("mask", [2, 128, B], U8, kind="ExternalInput")
    out_d = nc.dram_tensor("out", [B, H], F32, kind="ExternalOutput")

    with tile.TileContext(nc) as tc:
        with (
            tc.tile_pool(name="const", bufs=1) as cpool,
            tc.tile_pool(name="big", bufs=1) as bigpool,
            tc.tile_pool(name="xn", bufs=10) as xnpool,
            tc.tile_pool(name="small", bufs=2) as spool,
            tc.tile_pool(name="ps", bufs=8, space="PSUM") as pspool,
        ):
            # ---- constant loads -------------------------------------------
            w1t_sb = cpool.tile([128, KT, H], F16)
            ht_sb = cpool.tile([128, KT, B], F16)
            batt_sb = cpool.tile([128, JT], F32)
            v_sb = cpool.tile([128, JT], F16)
            mask_sb = cpool.tile([128, 2, B], U8)
            for k in range(KT):
                nc.sync.dma_start(w1t_sb[:, k, :], w1t_d[k * 128:(k + 1) * 128, :])
                nc.sync.dma_start(ht_sb[:, k, :], ht_d[k * 128:(k + 1) * 128, :])
            for j in range(JT):
                nc.sync.dma_start(batt_sb[:, j:j + 1], batt_d[j * 128:(j + 1) * 128, :])
                nc.sync.dma_start(v_sb[:, j:j + 1], v_d[j * 128:(j + 1) * 128, :])
            for sh in range(2):
                nc.sync.dma_start(mask_sb[:, sh, :], mask_d[sh])
            ident = cpool.tile([B, B], F32)
            masks.make_identity(nc, ident[:])

            # per-k tiles (not one fused tile) so the first energy matmuls
            # start as soon as k=0 lands instead of after the full 8MB
            w2t_ts, xt_ts = [], []
            for k in range(KT):
                w2k = cpool.tile([128, H], F16, tag=f"w2t{k}", name=f"w2t{k}")
                nc.sync.dma_start(w2k[:], w2t_d[k * 128:(k + 1) * 128, :])
                w2t_ts.append(w2k)
                xtk = bigpool.tile([128, TOK], F16, tag=f"xt{k}", name=f"xt{k}")
                nc.sync.dma_start(xtk[:], xt_d[k * 128:(k + 1) * 128, :])
                xt_ts.append(xtk)

            # ---- phase 1: bias[j,b] = b_att[j] + h[b] @ W1[j] -------------
            bias_sb = cpool.tile([128, JT, B], F32)
            for j in range(JT):
                ph = pspool.tile([128, B], F32, tag="ps")
                for k in range(KT):
                    nc.tensor.matmul(
                        ph[:], w1t_sb[:, k, j * 128:(j + 1) * 128], ht_sb[:, k, :],
                        start=(k == 0), stop=(k == KT - 1))
                nc.vector.tensor_scalar_add(bias_sb[:, j, :], ph[:], batt_sb[:, j:j + 1])

            # ---- phase 2: energyT = W2T.T @ XT; tanhE = tanh(. + bias) ----
            tanh_ts = []
            for j in range(JT):
                tj = bigpool.tile([128, TOK], F16, tag=f"tanh{j}", name=f"tanh{j}")
                tanh_ts.append(tj)
            for j in range(JT):
                pes = []
                for c in range(NCH):
                    pe_t = pspool.tile([128, 512], F32, tag="ps")
                    pes.append(pe_t)
                for k in range(KT):
                    lhs = w2t_ts[k][:, j * 128:(j + 1) * 128]
                    for c in range(NCH):
                        nc.tensor.matmul(
                            pes[c][:], lhs, xt_ts[k][:, c * 512:(c + 1) * 512],
                            start=(k == 0), stop=(k == KT - 1))
                for c in range(NCH):
                    for half in range(2):
                        b_idx = 2 * c + half
                        nc.scalar.activation(
                            tanh_ts[j][:, c * 512 + half * 256: c * 512 + (half + 1) * 256],
                            pes[c][:, half * 256:(half + 1) * 256],
                            AF.Tanh, bias=bias_sb[:, j, b_idx:b_idx + 1], scale=1.0)

            # ---- phase 3: score[t] = v . tanhE[:, t]  -> score_bs [B, SL] -
            score_bs = spool.tile([B, SL], F32, tag="score_bs")
            for c in range(NCH):
                ps_s = pspool.tile([1, 512], F32, tag="ps")
                for j in range(JT):
                    nc.tensor.matmul(
                        ps_s[:], v_sb[:, j:j + 1], tanh_ts[j][:, c * 512:(c + 1) * 512],
                        start=(j == 0), stop=(j == JT - 1))
                stg = spool.tile([1, 512], F32, tag="scst")
                nc.vector.tensor_copy(stg[:], ps_s[:])
                for half in range(2):
                    b_idx = 2 * c + half
                    nc.sync.dma_start(score_bs[b_idx:b_idx + 1, :],
                                      stg[:, half * 256:(half + 1) * 256])

            # ---- phase 4: mask + softmax over b -> w_sb [128, 2, B] f16 ---
            w_sb = spool.tile([128, 2, B], F16, tag="w_sb")
            for sh in range(2):
                ps_t = pspool.tile([128, B], F32, tag="ps")
                nc.tensor.transpose(ps_t[:], score_bs[:, sh * 128:(sh + 1) * 128], ident[:])
                mf = spool.tile([128, B], F32, tag="mf")
                nc.vector.tensor_copy(mf[:], mask_sb[:, sh, :])
                mbias = spool.tile([128, B], F32, tag="mbias")
                nc.vector.tensor_scalar(mbias[:], mf[:], 1e10, -1e10,
                                        op0=ALU.mult, op1=ALU.add)
                sm = spool.tile([128, B], F32, tag="sm")
                nc.vector.tensor_add(sm[:], ps_t[:], mbias[:])
                mx = spool.tile([128, 1], F32, tag="mx")
                nc.vector.reduce_max(mx[:], sm[:], axis=mybir.AxisListType.X)
                nmx = spool.tile([128, 1], F32, tag="nmx")
                nc.vector.tensor_scalar_mul(nmx[:], mx[:], -1.0)
                ex = spool.tile([128, B], F32, tag="ex")
                ssum = spool.tile([128, 1], F32, tag="ssum")
                nc.scalar.activation(ex[:], sm[:], AF.Exp,
                                     bias=nmx[:, 0:1], scale=1.0, accum_out=ssum[:])
                rs = spool.tile([128, 1], F32, tag="rs")
                nc.vector.reciprocal(rs[:], ssum[:])
                nc.vector.tensor_scalar_mul(w_sb[:, sh, :], ex[:], rs[:, 0:1])

            # ---- phase 5: context[b, :] = sum_s w[s,b] * hs[s,b,:] --------
            for b in range(B):
                xn_ts = []
                for sh in range(2):
                    t = xnpool.tile([128, H], F16, tag="xn")
                    nc.sync.dma_start(t[:], xn_d[sh * 128:(sh + 1) * 128, b, :])
                    xn_ts.append(t)
                for jc in range(2):
                    ps_c = pspool.tile([1, 512], F32, tag="ps")
                    for sh in range(2):
                        nc.tensor.matmul(
                            ps_c[:], w_sb[:, sh, b:b + 1],
                            xn_ts[sh][:, jc * 512:(jc + 1) * 512],
                            start=(sh == 0), stop=(sh == 1))
                    ostg = spool.tile([1, 512], F32, tag="ostg")
                    nc.vector.tensor_copy(ostg[:], ps_c[:])
                    nc.sync.dma_start(out_d[b:b + 1, jc * 512:(jc + 1) * 512], ostg[:])

    _split_multiwaits(nc)
    return nc


def _split_multiwaits(nc):
    """This walrus build encodes at most one sync-wait per regular instruction.

    Tile's sem-assigner can attach several. Hoist all-but-one wait onto fresh
    same-engine no-fuse NOPs placed immediately before the instruction —
    semantically identical, encodable.
    """
    import concourse.mybir as mybir
    n = 0
    for f in nc.m.functions:
        for bb in f.blocks:
            insts = bb.instructions
            new = []
            changed = False
            for inst in insts:
                si = getattr(inst, "sync_info", None)
                waits = list(si.on_wait) if (si and si.on_wait) else []
                if len(waits) > 1:
                    for w in waits[:-1]:
                        n += 1
                        new.append(mybir.InstNoOp(
                            name=f"I-wsplit-{n}", ins=[], outs=[],
                            engine=inst.engine,
                            sync_info=mybir.SyncInfo(on_wait=[w], on_update=[]),
                            bass_nofuse=True))
                    inst.sync_info = mybir.SyncInfo(
                        on_wait=[waits[-1]], on_update=list(si.on_update or []))
                    changed = True
                new.append(inst)
            if changed:
                bb.instructions = new
    return n


def _prep_inputs(hidden, hidden_sequence, input_masks, W_att, b_att, v):
    hidden = np.asarray(hidden, dtype=np.float32)
    hs = np.asarray(hidden_sequence, dtype=np.float32)
    mask = np.asarray(input_masks)
    W_att = np.asarray(W_att, dtype=np.float32)
    b_att = np.asarray(b_att, dtype=np.float32)
    v = np.asarray(v, dtype=np.float32)

    w1t = np.ascontiguousarray(W_att[:, :H].T).astype(np.float16)
    w2t = np.ascontiguousarray(W_att[:, H:].T).astype(np.float16)
    ht = np.ascontiguousarray(hidden[0].T).astype(np.float16)
    batt = b_att.reshape(H, 1).astype(np.float32)
    v16 = v.reshape(H, 1).astype(np.float16)

    in_maps = []
    for c in range(NCORES):
        hs_c = hs[c * SL:(c + 1) * SL]                       # [256, 16, H]
        xt = hs_c.transpose(2, 1, 0).reshape(H, TOK).astype(np.float16)
        xn = hs_c.astype(np.float16)
        m_c = mask[c * SL:(c + 1) * SL].astype(np.uint8).reshape(2, 128, B)
        in_maps.append({
            "xt": xt, "xn": xn, "w2t": w2t, "w1t": w1t, "ht": ht,
            "batt": batt, "v": v16, "mask": m_c,
        })
    return in_maps


def _run(inputs, trace=False, **trace_kwargs):
    from concourse.bass_utils import run_bass_kernel_spmd
    if "nc" not in _cached:
        _cached["nc"] = _build_nc()
    in_maps = _prep_inputs(**inputs)
    return run_bass_kernel_spmd(_cached["nc"], in_maps, list(range(NCORES)),
                                trace=trace, **trace_kwargs)


def kernel(hidden, hidden_sequence, input_masks, W_att, b_att, v):
    res = _run(dict(hidden=hidden, hidden_sequence=hidden_sequence,
                    input_masks=input_masks, W_att=W_att, b_att=b_att, v=v))
    parts = np.stack([r["out"] for r in res.results], axis=0)   # [8, B, H]
    ctx = parts.sum(axis=0, dtype=np.float64).astype(np.float32)
    return ctx[None, :, :]                                      # [1, B, H]
